# revision 36
# baseline (speedup 1.0000x reference)
"""Gabor-atom additive audio synthesis on 8 Trainium2 NeuronCores.

Math: waveform[t] = sum_n amp_n * exp(-0.5*((t-tau_n)/sigma_n)^2)
                    * cos(2*pi*omega_n*(t-tau_n) + gamma_n*(t-tau_n)^2 + phi_n)
with N=2048 atoms, T=48000 samples (2s @ 24kHz).

Sharding: atoms sorted by per-sample phase rate beta=omega_eff/fs across the
whole problem, dealt to 8 cores in runs of 256 (2 blocks of 128 partitions).
The sum over atoms is permutation invariant; sorting makes each block's
rate set compact.

Phase path: within a 768-sample tile, phase in radians is
y = 2*pi*(saw_m(j) + C + r*j) with m = round(768*beta) and
saw_m(j) = cfrac(m*j/768), a centered sawtooth whose period divides 768 —
the same 768-wide sawtooth rows serve every tile. A PE matmul selects each
atom's sawtooth row one-hot and adds per-tile C (2 bf16 limbs) and
residual-rate rows r*j (|r| <= 1/1536, 3 limb-product rows). With C
re-centered per (atom, tile) on host, |y| <= ~7.9 rad < 3*pi, so one DVE
ADD_RANGE_WRAP (in place in PSUM) lands the phase in [-pi, pi] and one ACT
Sin (the +pi/2 shift is folded into C so Sin yields the cosine) produces
the carrier. ACT runs a single table set: no activation-table swaps.

Envelope + reduction are fused into the PE: over each 128-sample output
chunk the envelope is linear, env ~= e0[chunk] + p*de[chunk] (p = position
in chunk = output PSUM partition), so the atom-sum reduce runs twice with
the cosine tile as stationary and the per-atom knot vectors e0/de as
moving: p_out[c] = sum_n e0_n*cos, q_out[c] = sum_n de_n*cos. The final
waveform is one tensor_scalar + add: wave = p_out + p*q_out. No
per-element envelope expansion or multiply exists anywhere.
Host: fp64 coefficient prep, final 8-way partial sum.
"""
import numpy as np
import ml_dtypes
from contextlib import ExitStack

import concourse.bacc as bacc
import concourse.tile as tile
from concourse import mybir
from concourse.bass_utils import run_bass_kernel_spmd

# ---- problem constants (hardcoded; kernel.py must be self-contained) ----
FS = 24000.0
T = 48000
N_ATOMS = 2048
N_CORES = 8
NYQUIST = FS / 2.0
SIGMA_OFFSET = 1e-3

P = 128                      # partitions / atoms per block
BLOCKS = 2                   # atom blocks per core (256 atoms/core)
F = 768                      # time-tile width; sawtooth periods divide F
N_TILES = T // F + (1 if T % F else 0)      # 63 (62 full + 384 remainder)
REM = T - (N_TILES - 1) * F                 # 384
CHUNK = 128                  # reduce chunk (output column) width
N_COLS = T // CHUNK          # 375 output columns
KSAW = 75                    # sawtooth one-hot rows per block (padded)
K = KSAW + 5                 # + C1, C2 (ones rows) and r1*j1, r1*j2, r2*j1
DMA_GRP = 8                  # stat DMA split: tiles per chunk

f32 = mybir.dt.float32
f16 = mybir.dt.float16
bf16 = mybir.dt.bfloat16
bft = ml_dtypes.bfloat16
TWO_PI = 2.0 * np.pi

_cache = {}


def _build_program():
    nc = bacc.Bacc("TRN2", target_bir_lowering=False, debug=False)

    d_saw = [nc.dram_tensor(f"saw{b}", [K, F], bf16, kind="ExternalInput").ap()
             for b in range(BLOCKS)]
    # stationary, k-major: [K, tile, block, atom]
    d_stat = nc.dram_tensor("stat", [K, N_TILES, BLOCKS, P], bf16,
                            kind="ExternalInput").ap()
    # envelope knots per block: value/slope per 128-sample chunk
    d_e0 = [nc.dram_tensor(f"e0_{b}", [P, N_COLS], bf16,
                           kind="ExternalInput").ap() for b in range(BLOCKS)]
    d_de = [nc.dram_tensor(f"de_{b}", [P, N_COLS], bf16,
                           kind="ExternalInput").ap() for b in range(BLOCKS)]
    d_prow = nc.dram_tensor("prow", [P, 1], f32, kind="ExternalInput").ap()
    d_out = nc.dram_tensor("wave", [P, N_COLS], f32, kind="ExternalOutput").ap()

    with tile.TileContext(nc) as tc, ExitStack() as ctx:
        consts = ctx.enter_context(tc.tile_pool(name="consts", bufs=1))
        phpool = ctx.enter_context(tc.tile_pool(name="ph", bufs=8))
        cospool = ctx.enter_context(tc.tile_pool(name="cos", bufs=14))
        yfpool = ctx.enter_context(tc.tile_pool(name="yf", bufs=2))
        kpool = ctx.enter_context(tc.tile_pool(name="k2pi", bufs=2))
        opool = ctx.enter_context(tc.tile_pool(name="ocopy", bufs=1))
        ypool = ctx.enter_context(tc.tile_pool(name="yp", bufs=2, space="PSUM"))
        outpool = ctx.enter_context(tc.tile_pool(name="outp", bufs=1,
                                                 space="PSUM"))

        # ---- resident constants ----
        t_saw = []
        for b in range(BLOCKS):
            t = consts.tile([K, F], bf16, tag=f"saw{b}")
            nc.sync.dma_start(t[:], d_saw[b][:])
            t_saw.append(t)
        t_e0, t_de = [], []
        for b in range(BLOCKS):
            te = consts.tile([P, N_COLS], bf16, tag=f"e0_{b}")
            nc.gpsimd.dma_start(te[:], d_e0[b][:])
            t_e0.append(te)
            td = consts.tile([P, N_COLS], bf16, tag=f"de_{b}")
            nc.gpsimd.dma_start(td[:], d_de[b][:])
            t_de.append(td)
        t_prow = consts.tile([P, 1], f32, tag="prow")
        nc.gpsimd.dma_start(t_prow[:], d_prow[:])

        # all per-tile stationaries, resident. The first chunk covers only
        # the first tiles processed (fast pipeline start), the rest follow
        # in DMA_GRP-tile chunks.
        t_stat = consts.tile([K, N_TILES * BLOCKS * P], bf16, tag="stat")

        def stat_dma(lo, hi):
            nc.sync.dma_start(
                t_stat[:, lo * BLOCKS * P: hi * BLOCKS * P],
                d_stat[:, lo:hi])

        stat_dma(N_TILES - 1, N_TILES)   # REM tile, processed first
        stat_dma(0, 2)
        i = 2
        while i < N_TILES - 1:
            hi = min(i + DMA_GRP, N_TILES - 1)
            stat_dma(i, hi)
            i = hi

        p_out = outpool.tile([P, 512], f32, tag="po")
        q_out = outpool.tile([P, 512], f32, tag="qo")

        def tf(i):
            return REM if i == N_TILES - 1 else F

        def spans_of(w):
            return ([slice(0, BLOCKS * F)] if w == F else
                    [slice(b * F, b * F + w) for b in range(BLOCKS)])

        # two-stage software pipeline: stage_a(i) produces the cosine tile;
        # the weighted reduce runs one tile behind so PE queue heads always
        # have ready deps. Quarter-angle tiles skip the DVE wrap entirely:
        # Sin(0.25*y) is always in range (|y|/4 <= ~2 < pi), and
        # cos(y) = 2*(1 - 2*sin^2(y/4))^2 - 1 reconstructs via two ACT
        # Squares (same table set as Sin) staged over later iterations plus
        # two cheap 4x DVE tensor_scalars.
        stash = {}
        qstate = {}
        QUARTER_TILES = frozenset((16, 28, 40, 52))

        def stage_a(i):
            w = tf(i)
            p_y = ypool.tile([P, BLOCKS * F], f32, tag="y")
            for b in range(BLOCKS):
                st = t_stat[:, (i * BLOCKS + b) * P: (i * BLOCKS + b + 1) * P]
                o = 0
                while o < w:
                    col = b * F + o
                    n = min(w - o, 512 - (col % 512))
                    nc.tensor.matmul(p_y[:, col: col + n],
                                     st, t_saw[b][:, o:o + n],
                                     start=True, stop=True)
                    o += n
            t_cos = cospool.tile([P, BLOCKS * F], bf16, tag="cos")
            if i in QUARTER_TILES:
                t_s = yfpool.tile([P, BLOCKS * F], f16, tag="qs")
                for sl in spans_of(w):
                    nc.scalar.activation(t_s[:, sl], p_y[:, sl],
                                         mybir.ActivationFunctionType.Sin,
                                         scale=0.25)
                qstate[i] = (t_s, t_cos, w)
            else:
                # phase wrap into [-pi, pi], PSUM -> SBUF f16 so the PSUM
                # tile frees at the wrap itself and the pipeline runs at
                # wrap rate (one period is enough: host centering keeps
                # |y| <= ~7.9 < 3*pi)
                t_ph = phpool.tile([P, BLOCKS * F], f16, tag="ph")
                for sl in spans_of(w):
                    nc.vector.add_range_wrap(t_ph[:, sl], p_y[:, sl],
                                             0.0, np.pi, TWO_PI)
                    nc.scalar.activation(t_cos[:, sl], t_ph[:, sl],
                                         mybir.ActivationFunctionType.Sin)
            stash[i] = t_cos

        def stage_q1(i):
            t_s, t_cos, w = qstate[i]
            t_g = kpool.tile([P, BLOCKS * F], f16, tag="qg")
            for sl in spans_of(w):
                nc.scalar.activation(t_s[:, sl], t_s[:, sl],
                                     mybir.ActivationFunctionType.Square)
                nc.vector.tensor_scalar(t_g[:, sl], t_s[:, sl], -2.0, 1.0,
                                        mybir.AluOpType.mult,
                                        mybir.AluOpType.add)
            qstate[i] = (t_g, t_cos, w)

        def stage_q2(i):
            t_g, t_cos, w = qstate.pop(i)
            for sl in spans_of(w):
                nc.scalar.activation(t_g[:, sl], t_g[:, sl],
                                     mybir.ActivationFunctionType.Square)
                nc.vector.tensor_scalar(t_cos[:, sl], t_g[:, sl], 2.0, -1.0,
                                        mybir.AluOpType.mult,
                                        mybir.AluOpType.add)

        def stage_reduce(i):
            w = tf(i)
            t_cos = stash.pop(i)
            for j in range(w // CHUNK):
                c = (i * F) // CHUNK + j
                for dst, mov in ((p_out, t_e0), (q_out, t_de)):
                    for b in range(BLOCKS):
                        nc.tensor.matmul(
                            dst[:, c:c + 1],
                            t_cos[:, b * F + j * CHUNK: b * F + (j + 1) * CHUNK],
                            mov[b][:, c:c + 1],
                            start=(b == 0), stop=(b == BLOCKS - 1))

        # process the small REM tile first: faster pipeline fill and the
        # run ends on a predictable full tile. Quarter tiles are swapped
        # ahead of their predecessor so their PSUM-reading Sin heads ACT's
        # queue and frees the PSUM slot without waiting a tile of Sins.
        order = [N_TILES - 1] + list(range(N_TILES - 1))
        for q in QUARTER_TILES:
            k = order.index(q)
            order[k - 1], order[k] = order[k], order[k - 1]
        t_t = opool.tile([P, N_COLS], f32, tag="tmp")
        t_w = opool.tile([P, N_COLS], f32, tag="w")

        def combine(c_lo, c_hi):
            # wave = p_out + prow*q_out (env = e0 + p*de within each chunk)
            cs = slice(c_lo, c_hi)
            nc.vector.tensor_scalar(t_t[:, cs], q_out[:, cs], t_prow[:],
                                    None, mybir.AluOpType.mult)
            nc.vector.tensor_tensor(t_w[:, cs], t_t[:, cs], p_out[:, cs],
                                    mybir.AluOpType.add)
            nc.sync.dma_start(d_out[:, cs], t_w[:, cs])

        for k, i in enumerate(order):
            stage_a(i)
            if k >= 2 and order[k - 2] in QUARTER_TILES:
                stage_q1(order[k - 2])
            if k >= 4 and order[k - 4] in QUARTER_TILES:
                stage_q2(order[k - 4])
            if k >= 3 and order[k - 3] not in QUARTER_TILES:
                stage_reduce(order[k - 3])
            if k >= 6 and order[k - 6] in QUARTER_TILES:
                stage_reduce(order[k - 6])
            if k == 40:
                # tiles 0..29 (cols 0..179) are reduced by now: overlap the
                # first chunk of the final combine + output DMA
                combine(0, 180)
        for k in range(len(order) - 3, len(order)):
            if order[k] in stash:
                stage_reduce(order[k])
        assert not stash and not qstate
        combine(180, N_COLS)

    nc.compile()
    return nc


def _cfrac(x):
    return x - np.round(x)


def _prepare_inputs(amplitude_logit, tau, omega_logit, sigma_logit,
                    phi_vector, gamma):
    """fp64 host prep -> per-core input maps."""
    al = amplitude_logit.astype(np.float64)
    tau = tau.astype(np.float64)
    ol = omega_logit.astype(np.float64)
    sl = sigma_logit.astype(np.float64)
    pv = phi_vector.astype(np.float64)
    gamma = gamma.astype(np.float64)

    amp = np.where(al > 30, al, np.log1p(np.exp(al)))
    omega = (1.0 / (1.0 + np.exp(-ol))) * 0.99 * NYQUIST
    sigma = np.where(sl > 30, sl, np.log1p(np.exp(sl))) + SIGMA_OFFSET
    phi = np.arctan2(pv[:, 1], pv[:, 0])

    # sort atoms by center phase rate (cycles/sample); deal runs of 256/core
    beta_mid = (omega + gamma * (1.0 - tau) / np.pi) / FS
    order = np.argsort(beta_mid)
    amp, tau_s, omega_s = amp[order], tau[order], omega[order]
    sigma_s, phi_s, gamma_s = sigma[order], phi[order], gamma[order]
    m_all = np.round(F * beta_mid[order]).astype(np.int64)

    jl = np.arange(F, dtype=np.float64)
    t0s = np.arange(N_TILES, dtype=np.float64) * F / FS        # [I]
    kn = np.arange(N_COLS + 1, dtype=np.float64) * CHUNK / FS  # chunk knots
    prow = np.arange(P, dtype=np.float32).reshape(P, 1)

    in_maps = []
    for c in range(N_CORES):
        saws, e0s, des = [], [], []
        stat = np.zeros((K, N_TILES, BLOCKS, P), dtype=bft)
        for b in range(BLOCKS):
            sel = slice(c * BLOCKS * P + b * P, c * BLOCKS * P + (b + 1) * P)
            am, ta, om = amp[sel], tau_s[sel], omega_s[sel]
            sg, ph, ga = sigma_s[sel], phi_s[sel], gamma_s[sel]
            m = m_all[sel]

            ms = np.unique(m)
            ms = ms[ms != 0]
            assert len(ms) <= KSAW, f"block saw rows {len(ms)} > {KSAW}"
            saw = np.zeros((K, F), dtype=bft)
            saw[:len(ms)] = (TWO_PI * _cfrac(ms[:, None] * jl[None, :] / F)
                             ).astype(bft)
            # residual-rate moving rows: ones, ones, j1, j2, j1
            j1 = jl.astype(bft)
            j2 = (jl - j1.astype(np.float64)).astype(bft)
            saw[KSAW + 0] = bft(1.0)
            saw[KSAW + 1] = bft(1.0)
            saw[KSAW + 2] = j1
            saw[KSAW + 3] = j2
            saw[KSAW + 4] = j1
            saws.append(np.ascontiguousarray(saw))

            row_of = {mm: r for r, mm in enumerate(ms)}
            rows = np.array([row_of.get(mm, -1) for mm in m])  # [P]

            # per (tile, atom): phase at tile start, rate, residual
            D = t0s[:, None] - ta[None, :]                      # [I, P]
            Y0 = (om[None, :] * D + ga[None, :] * D * D / TWO_PI
                  + ph[None, :] / TWO_PI + 0.25)               # cycles
            beta_t = (om[None, :] + ga[None, :] * D / np.pi) / FS
            r = beta_t - m[None, :] / F                        # [I, P]
            assert np.abs(r).max() < 1.0 / 1536 + 1e-5

            C_raw = _cfrac(Y0)
            mid = C_raw + r * (F - 1) / 2.0
            Cc = C_raw - np.round(mid)                         # center |y|
            C_rad = TWO_PI * Cc
            C1 = C_rad.astype(bft)
            C2 = (C_rad - C1.astype(np.float64)).astype(bft)
            r_rad = TWO_PI * r
            r1 = r_rad.astype(bft)
            r2 = (r_rad - r1.astype(np.float64)).astype(bft)

            onehot = np.zeros((KSAW, P), dtype=bft)
            pidx = np.nonzero(rows >= 0)[0]
            onehot[rows[pidx], pidx] = bft(1.0)
            stat[:KSAW, :, b, :] = onehot[:, None, :]
            stat[KSAW + 0, :, b, :] = C1
            stat[KSAW + 1, :, b, :] = C2
            stat[KSAW + 2, :, b, :] = r1
            stat[KSAW + 3, :, b, :] = r1
            stat[KSAW + 4, :, b, :] = r2

            # envelope knots at chunk boundaries -> e0, de per chunk
            dk = kn[:, None] - ta[None, :]                      # [C+1, P]
            ev = am[None, :] * np.exp(-0.5 * (dk / sg[None, :]) ** 2)
            e0 = ev[:-1]                                        # [C, P]
            de = (ev[1:] - ev[:-1]) / CHUNK
            e0s.append(np.ascontiguousarray(e0.T.astype(bft)))
            des.append(np.ascontiguousarray(de.T.astype(bft)))

        im = {"stat": np.ascontiguousarray(stat),
              "prow": prow}
        for b in range(BLOCKS):
            im[f"saw{b}"] = saws[b]
            im[f"e0_{b}"] = e0s[b]
            im[f"de_{b}"] = des[b]
        in_maps.append(im)
    return in_maps


def kernel(amplitude_logit, tau, omega_logit, sigma_logit, phi_vector, gamma, t):
    if "nc" not in _cache:
        _cache["nc"] = _build_program()
    nc = _cache["nc"]
    in_maps = _prepare_inputs(amplitude_logit, tau, omega_logit, sigma_logit,
                              phi_vector, gamma)
    res = run_bass_kernel_spmd(nc, in_maps, list(range(N_CORES)))
    total = np.zeros(T, dtype=np.float64)
    for r in res.results:
        wv = r["wave"].astype(np.float64)          # [P, N_COLS]
        total += wv.T.ravel()                      # sample s = c*128 + p
    return total.astype(np.float32)


# revision 40
# speedup vs baseline: 1.0106x; 1.0106x over previous
"""Gabor-atom additive audio synthesis on 8 Trainium2 NeuronCores.

Math: waveform[t] = sum_n amp_n * exp(-0.5*((t-tau_n)/sigma_n)^2)
                    * cos(2*pi*omega_n*(t-tau_n) + gamma_n*(t-tau_n)^2 + phi_n)
with N=2048 atoms, T=48000 samples (2s @ 24kHz).

Sharding: atoms sorted by per-sample phase rate beta=omega_eff/fs across the
whole problem, dealt to 8 cores in runs of 256 (2 blocks of 128 partitions).
The sum over atoms is permutation invariant; sorting makes each block's
rate set compact.

Phase path: within a 768-sample tile, phase in radians is
y = 2*pi*(saw_m(j) + C + r*j) with m = round(768*beta) and
saw_m(j) = cfrac(m*j/768), a centered sawtooth whose period divides 768 —
the same 768-wide sawtooth rows serve every tile. A PE matmul selects each
atom's sawtooth row one-hot and adds per-tile C (2 bf16 limbs) and
residual-rate rows r*j (|r| <= 1/1536, 3 limb-product rows). With C
re-centered per (atom, tile) on host, |y| <= ~7.9 rad < 3*pi, so one DVE
ADD_RANGE_WRAP (in place in PSUM) lands the phase in [-pi, pi] and one ACT
Sin (the +pi/2 shift is folded into C so Sin yields the cosine) produces
the carrier. ACT runs a single table set: no activation-table swaps.

Envelope + reduction are fused into the PE: over each 128-sample output
chunk the envelope is linear, env ~= e0[chunk] + p*de[chunk] (p = position
in chunk = output PSUM partition), so the atom-sum reduce runs twice with
the cosine tile as stationary and the per-atom knot vectors e0/de as
moving: p_out[c] = sum_n e0_n*cos, q_out[c] = sum_n de_n*cos. The final
waveform is one tensor_scalar + add: wave = p_out + p*q_out. No
per-element envelope expansion or multiply exists anywhere.
Host: fp64 coefficient prep, final 8-way partial sum.
"""
import numpy as np
import ml_dtypes
from contextlib import ExitStack

import concourse.bacc as bacc
import concourse.tile as tile
from concourse import mybir
from concourse.bass_utils import run_bass_kernel_spmd

# ---- problem constants (hardcoded; kernel.py must be self-contained) ----
FS = 24000.0
T = 48000
N_ATOMS = 2048
N_CORES = 8
NYQUIST = FS / 2.0
SIGMA_OFFSET = 1e-3

P = 128                      # partitions / atoms per block
BLOCKS = 2                   # atom blocks per core (256 atoms/core)
F = 768                      # time-tile width; sawtooth periods divide F
N_TILES = T // F + (1 if T % F else 0)      # 63 (62 full + 384 remainder)
REM = T - (N_TILES - 1) * F                 # 384
CHUNK = 128                  # reduce chunk (output column) width
N_COLS = T // CHUNK          # 375 output columns
KSAW = 75                    # sawtooth one-hot rows per block (padded)
K = KSAW + 5                 # + C1, C2 (ones rows) and r1*j1, r1*j2, r2*j1
DMA_GRP = 8                  # stat DMA split: tiles per chunk

f32 = mybir.dt.float32
f16 = mybir.dt.float16
bf16 = mybir.dt.bfloat16
bft = ml_dtypes.bfloat16
TWO_PI = 2.0 * np.pi

_cache = {}


def _build_program():
    nc = bacc.Bacc("TRN2", target_bir_lowering=False, debug=False)

    d_saw = [nc.dram_tensor(f"saw{b}", [K, F], bf16, kind="ExternalInput").ap()
             for b in range(BLOCKS)]
    # stationary, k-major: [K, tile, block, atom]
    d_stat = nc.dram_tensor("stat", [K, N_TILES, BLOCKS, P], bf16,
                            kind="ExternalInput").ap()
    # envelope knots per block: value/slope per 128-sample chunk
    d_e0 = [nc.dram_tensor(f"e0_{b}", [P, N_COLS], bf16,
                           kind="ExternalInput").ap() for b in range(BLOCKS)]
    d_de = [nc.dram_tensor(f"de_{b}", [P, N_COLS], bf16,
                           kind="ExternalInput").ap() for b in range(BLOCKS)]
    d_prow = nc.dram_tensor("prow", [P, 1], f32, kind="ExternalInput").ap()
    d_out = nc.dram_tensor("wave", [P, N_COLS], f32, kind="ExternalOutput").ap()

    with tile.TileContext(nc) as tc, ExitStack() as ctx:
        consts = ctx.enter_context(tc.tile_pool(name="consts", bufs=1))
        phpool = ctx.enter_context(tc.tile_pool(name="ph", bufs=8))
        cospool = ctx.enter_context(tc.tile_pool(name="cos", bufs=14))
        yfpool = ctx.enter_context(tc.tile_pool(name="yf", bufs=2))
        kpool = ctx.enter_context(tc.tile_pool(name="k2pi", bufs=2))
        opool = ctx.enter_context(tc.tile_pool(name="ocopy", bufs=1))
        ypool = ctx.enter_context(tc.tile_pool(name="yp", bufs=2, space="PSUM"))
        outpool = ctx.enter_context(tc.tile_pool(name="outp", bufs=1,
                                                 space="PSUM"))

        # ---- resident constants (queues spread so fixed DMA overheads
        # overlap during the pipeline ramp) ----
        t_saw = []
        for b, eng in zip(range(BLOCKS), (nc.sync, nc.scalar)):
            t = consts.tile([K, F], bf16, tag=f"saw{b}")
            eng.dma_start(t[:], d_saw[b][:])
            t_saw.append(t)
        t_e0, t_de = [], []
        for b in range(BLOCKS):
            te = consts.tile([P, N_COLS], bf16, tag=f"e0_{b}")
            nc.gpsimd.dma_start(te[:], d_e0[b][:])
            t_e0.append(te)
            td = consts.tile([P, N_COLS], bf16, tag=f"de_{b}")
            nc.gpsimd.dma_start(td[:], d_de[b][:])
            t_de.append(td)
        t_prow = consts.tile([P, 1], f32, tag="prow")
        nc.gpsimd.dma_start(t_prow[:], d_prow[:])

        # all per-tile stationaries, resident. The first chunk covers only
        # the first tiles processed (fast pipeline start), the rest follow
        # in DMA_GRP-tile chunks.
        t_stat = consts.tile([K, N_TILES * BLOCKS * P], bf16, tag="stat")

        def stat_dma(lo, hi):
            nc.sync.dma_start(
                t_stat[:, lo * BLOCKS * P: hi * BLOCKS * P],
                d_stat[:, lo:hi])

        nc.scalar.dma_start(
            t_stat[:, (N_TILES - 1) * BLOCKS * P:],
            d_stat[:, N_TILES - 1:])         # REM tile, processed first
        stat_dma(0, 2)
        i = 2
        while i < N_TILES - 1:
            hi = min(i + DMA_GRP, N_TILES - 1)
            stat_dma(i, hi)
            i = hi

        p_out = outpool.tile([P, 512], f32, tag="po")
        q_out = outpool.tile([P, 512], f32, tag="qo")

        def tf(i):
            return REM if i == N_TILES - 1 else F

        def spans_of(w):
            return ([slice(0, BLOCKS * F)] if w == F else
                    [slice(b * F, b * F + w) for b in range(BLOCKS)])

        # two-stage software pipeline: stage_a(i) produces the cosine tile;
        # the weighted reduce runs one tile behind so PE queue heads always
        # have ready deps. Quarter-angle tiles skip the DVE wrap entirely:
        # Sin(0.25*y) is always in range (|y|/4 <= ~2 < pi), and
        # cos(y) = 2*(1 - 2*sin^2(y/4))^2 - 1 reconstructs via two ACT
        # Squares (same table set as Sin) staged over later iterations plus
        # two cheap 4x DVE tensor_scalars.
        stash = {}
        qstate = {}
        QUARTER_TILES = frozenset()

        def stage_a(i):
            w = tf(i)
            p_y = ypool.tile([P, BLOCKS * F], f32, tag="y")
            for b in range(BLOCKS):
                st = t_stat[:, (i * BLOCKS + b) * P: (i * BLOCKS + b + 1) * P]
                o = 0
                while o < w:
                    col = b * F + o
                    n = min(w - o, 512 - (col % 512))
                    nc.tensor.matmul(p_y[:, col: col + n],
                                     st, t_saw[b][:, o:o + n],
                                     start=True, stop=True)
                    o += n
            t_cos = cospool.tile([P, BLOCKS * F], bf16, tag="cos")
            if i in QUARTER_TILES:
                t_s = yfpool.tile([P, BLOCKS * F], f16, tag="qs")
                for sl in spans_of(w):
                    nc.scalar.activation(t_s[:, sl], p_y[:, sl],
                                         mybir.ActivationFunctionType.Sin,
                                         scale=0.25)
                qstate[i] = (t_s, t_cos, w)
            else:
                # phase wrap into [-pi, pi], PSUM -> SBUF f16 so the PSUM
                # tile frees at the wrap itself and the pipeline runs at
                # wrap rate (one period is enough: host centering keeps
                # |y| <= ~7.9 < 3*pi)
                t_ph = phpool.tile([P, BLOCKS * F], f16, tag="ph")
                for sl in spans_of(w):
                    nc.vector.add_range_wrap(t_ph[:, sl], p_y[:, sl],
                                             0.0, np.pi, TWO_PI)
                    nc.scalar.activation(t_cos[:, sl], t_ph[:, sl],
                                         mybir.ActivationFunctionType.Sin)
            stash[i] = t_cos

        def stage_q1(i):
            t_s, t_cos, w = qstate[i]
            t_g = kpool.tile([P, BLOCKS * F], f16, tag="qg")
            for sl in spans_of(w):
                nc.scalar.activation(t_s[:, sl], t_s[:, sl],
                                     mybir.ActivationFunctionType.Square)
                nc.vector.tensor_scalar(t_g[:, sl], t_s[:, sl], -2.0, 1.0,
                                        mybir.AluOpType.mult,
                                        mybir.AluOpType.add)
            qstate[i] = (t_g, t_cos, w)

        def stage_q2(i):
            t_g, t_cos, w = qstate.pop(i)
            for sl in spans_of(w):
                nc.scalar.activation(t_g[:, sl], t_g[:, sl],
                                     mybir.ActivationFunctionType.Square)
                nc.vector.tensor_scalar(t_cos[:, sl], t_g[:, sl], 2.0, -1.0,
                                        mybir.AluOpType.mult,
                                        mybir.AluOpType.add)

        def stage_reduce(i):
            w = tf(i)
            t_cos = stash.pop(i)
            for j in range(w // CHUNK):
                c = (i * F) // CHUNK + j
                for dst, mov in ((p_out, t_e0), (q_out, t_de)):
                    for b in range(BLOCKS):
                        nc.tensor.matmul(
                            dst[:, c:c + 1],
                            t_cos[:, b * F + j * CHUNK: b * F + (j + 1) * CHUNK],
                            mov[b][:, c:c + 1],
                            start=(b == 0), stop=(b == BLOCKS - 1))

        # process the small REM tile first: faster pipeline fill and the
        # run ends on a predictable full tile. Quarter tiles are swapped
        # ahead of their predecessor so their PSUM-reading Sin heads ACT's
        # queue and frees the PSUM slot without waiting a tile of Sins.
        order = [N_TILES - 1] + list(range(N_TILES - 1))
        for q in QUARTER_TILES:
            k = order.index(q)
            order[k - 1], order[k] = order[k], order[k - 1]
        t_t = opool.tile([P, N_COLS], f32, tag="tmp")
        t_w = opool.tile([P, N_COLS], f32, tag="w")

        def combine(c_lo, c_hi):
            # wave = p_out + prow*q_out (env = e0 + p*de within each chunk)
            cs = slice(c_lo, c_hi)
            nc.vector.tensor_scalar(t_t[:, cs], q_out[:, cs], t_prow[:],
                                    None, mybir.AluOpType.mult)
            nc.vector.tensor_tensor(t_w[:, cs], t_t[:, cs], p_out[:, cs],
                                    mybir.AluOpType.add)
            nc.sync.dma_start(d_out[:, cs], t_w[:, cs])

        for k, i in enumerate(order):
            stage_a(i)
            if k >= 2 and order[k - 2] in QUARTER_TILES:
                stage_q1(order[k - 2])
            if k >= 4 and order[k - 4] in QUARTER_TILES:
                stage_q2(order[k - 4])
            if k >= 3 and order[k - 3] not in QUARTER_TILES:
                stage_reduce(order[k - 3])
            if k >= 6 and order[k - 6] in QUARTER_TILES:
                stage_reduce(order[k - 6])
            if k == 40:
                # tiles 0..29 (cols 0..179) are reduced by now: overlap the
                # first chunk of the final combine + output DMA
                combine(0, 180)
        for k in range(len(order) - 3, len(order)):
            if order[k] in stash:
                stage_reduce(order[k])
        assert not stash and not qstate
        combine(180, N_COLS)

    nc.compile()
    return nc


def _cfrac(x):
    return x - np.round(x)


def _prepare_inputs(amplitude_logit, tau, omega_logit, sigma_logit,
                    phi_vector, gamma):
    """fp64 host prep -> per-core input maps."""
    al = amplitude_logit.astype(np.float64)
    tau = tau.astype(np.float64)
    ol = omega_logit.astype(np.float64)
    sl = sigma_logit.astype(np.float64)
    pv = phi_vector.astype(np.float64)
    gamma = gamma.astype(np.float64)

    amp = np.where(al > 30, al, np.log1p(np.exp(al)))
    omega = (1.0 / (1.0 + np.exp(-ol))) * 0.99 * NYQUIST
    sigma = np.where(sl > 30, sl, np.log1p(np.exp(sl))) + SIGMA_OFFSET
    phi = np.arctan2(pv[:, 1], pv[:, 0])

    # sort atoms by center phase rate (cycles/sample); deal runs of 256/core
    beta_mid = (omega + gamma * (1.0 - tau) / np.pi) / FS
    order = np.argsort(beta_mid)
    amp, tau_s, omega_s = amp[order], tau[order], omega[order]
    sigma_s, phi_s, gamma_s = sigma[order], phi[order], gamma[order]
    m_all = np.round(F * beta_mid[order]).astype(np.int64)

    jl = np.arange(F, dtype=np.float64)
    t0s = np.arange(N_TILES, dtype=np.float64) * F / FS        # [I]
    kn = np.arange(N_COLS + 1, dtype=np.float64) * CHUNK / FS  # chunk knots
    prow = np.arange(P, dtype=np.float32).reshape(P, 1)

    in_maps = []
    for c in range(N_CORES):
        saws, e0s, des = [], [], []
        stat = np.zeros((K, N_TILES, BLOCKS, P), dtype=bft)
        for b in range(BLOCKS):
            sel = slice(c * BLOCKS * P + b * P, c * BLOCKS * P + (b + 1) * P)
            am, ta, om = amp[sel], tau_s[sel], omega_s[sel]
            sg, ph, ga = sigma_s[sel], phi_s[sel], gamma_s[sel]
            m = m_all[sel]

            ms = np.unique(m)
            ms = ms[ms != 0]
            assert len(ms) <= KSAW, f"block saw rows {len(ms)} > {KSAW}"
            saw = np.zeros((K, F), dtype=bft)
            saw[:len(ms)] = (TWO_PI * _cfrac(ms[:, None] * jl[None, :] / F)
                             ).astype(bft)
            # residual-rate moving rows: ones, ones, j1, j2, j1
            j1 = jl.astype(bft)
            j2 = (jl - j1.astype(np.float64)).astype(bft)
            saw[KSAW + 0] = bft(1.0)
            saw[KSAW + 1] = bft(1.0)
            saw[KSAW + 2] = j1
            saw[KSAW + 3] = j2
            saw[KSAW + 4] = j1
            saws.append(np.ascontiguousarray(saw))

            row_of = {mm: r for r, mm in enumerate(ms)}
            rows = np.array([row_of.get(mm, -1) for mm in m])  # [P]

            # per (tile, atom): phase at tile start, rate, residual
            D = t0s[:, None] - ta[None, :]                      # [I, P]
            Y0 = (om[None, :] * D + ga[None, :] * D * D / TWO_PI
                  + ph[None, :] / TWO_PI + 0.25)               # cycles
            beta_t = (om[None, :] + ga[None, :] * D / np.pi) / FS
            r = beta_t - m[None, :] / F                        # [I, P]
            assert np.abs(r).max() < 1.0 / 1536 + 1e-5

            C_raw = _cfrac(Y0)
            mid = C_raw + r * (F - 1) / 2.0
            Cc = C_raw - np.round(mid)                         # center |y|
            C_rad = TWO_PI * Cc
            C1 = C_rad.astype(bft)
            C2 = (C_rad - C1.astype(np.float64)).astype(bft)
            r_rad = TWO_PI * r
            r1 = r_rad.astype(bft)
            r2 = (r_rad - r1.astype(np.float64)).astype(bft)

            onehot = np.zeros((KSAW, P), dtype=bft)
            pidx = np.nonzero(rows >= 0)[0]
            onehot[rows[pidx], pidx] = bft(1.0)
            stat[:KSAW, :, b, :] = onehot[:, None, :]
            stat[KSAW + 0, :, b, :] = C1
            stat[KSAW + 1, :, b, :] = C2
            stat[KSAW + 2, :, b, :] = r1
            stat[KSAW + 3, :, b, :] = r1
            stat[KSAW + 4, :, b, :] = r2

            # envelope knots at chunk boundaries -> e0, de per chunk
            dk = kn[:, None] - ta[None, :]                      # [C+1, P]
            ev = am[None, :] * np.exp(-0.5 * (dk / sg[None, :]) ** 2)
            e0 = ev[:-1]                                        # [C, P]
            de = (ev[1:] - ev[:-1]) / CHUNK
            e0s.append(np.ascontiguousarray(e0.T.astype(bft)))
            des.append(np.ascontiguousarray(de.T.astype(bft)))

        im = {"stat": np.ascontiguousarray(stat),
              "prow": prow}
        for b in range(BLOCKS):
            im[f"saw{b}"] = saws[b]
            im[f"e0_{b}"] = e0s[b]
            im[f"de_{b}"] = des[b]
        in_maps.append(im)
    return in_maps


def kernel(amplitude_logit, tau, omega_logit, sigma_logit, phi_vector, gamma, t):
    if "nc" not in _cache:
        _cache["nc"] = _build_program()
    nc = _cache["nc"]
    in_maps = _prepare_inputs(amplitude_logit, tau, omega_logit, sigma_logit,
                              phi_vector, gamma)
    res = run_bass_kernel_spmd(nc, in_maps, list(range(N_CORES)))
    total = np.zeros(T, dtype=np.float64)
    for r in res.results:
        wv = r["wave"].astype(np.float64)          # [P, N_COLS]
        total += wv.T.ravel()                      # sample s = c*128 + p
    return total.astype(np.float32)


# revision 42
# speedup vs baseline: 1.0217x; 1.0110x over previous
"""Gabor-atom additive audio synthesis on 8 Trainium2 NeuronCores.

Math: waveform[t] = sum_n amp_n * exp(-0.5*((t-tau_n)/sigma_n)^2)
                    * cos(2*pi*omega_n*(t-tau_n) + gamma_n*(t-tau_n)^2 + phi_n)
with N=2048 atoms, T=48000 samples (2s @ 24kHz).

Sharding: atoms sorted by per-sample phase rate beta=omega_eff/fs across the
whole problem, dealt to 8 cores in runs of 256 (2 blocks of 128 partitions).
The sum over atoms is permutation invariant; sorting makes each block's
rate set compact.

Phase path: within a 768-sample tile, phase in radians is
y = 2*pi*(saw_m(j) + C + r*j) with m = round(768*beta) and
saw_m(j) = cfrac(m*j/768), a centered sawtooth whose period divides 768 —
the same 768-wide sawtooth rows serve every tile. A PE matmul selects each
atom's sawtooth row one-hot and adds per-tile C (2 bf16 limbs) and
residual-rate rows r*j (|r| <= 1/1536, 3 limb-product rows). With C
re-centered per (atom, tile) on host, |y| <= ~7.9 rad < 3*pi, so one DVE
ADD_RANGE_WRAP (in place in PSUM) lands the phase in [-pi, pi] and one ACT
Sin (the +pi/2 shift is folded into C so Sin yields the cosine) produces
the carrier. ACT runs a single table set: no activation-table swaps.

Envelope + reduction are fused into the PE: over each 128-sample output
chunk the envelope is linear, env ~= e0[chunk] + p*de[chunk] (p = position
in chunk = output PSUM partition), so the atom-sum reduce runs twice with
the cosine tile as stationary and the per-atom knot vectors e0/de as
moving: p_out[c] = sum_n e0_n*cos, q_out[c] = sum_n de_n*cos. The final
waveform is one tensor_scalar + add: wave = p_out + p*q_out. No
per-element envelope expansion or multiply exists anywhere.
Host: fp64 coefficient prep, final 8-way partial sum.
"""
import numpy as np
import ml_dtypes
from contextlib import ExitStack

import concourse.bacc as bacc
import concourse.tile as tile
from concourse import mybir
from concourse.bass_utils import run_bass_kernel_spmd

# ---- problem constants (hardcoded; kernel.py must be self-contained) ----
FS = 24000.0
T = 48000
N_ATOMS = 2048
N_CORES = 8
NYQUIST = FS / 2.0
SIGMA_OFFSET = 1e-3

P = 128                      # partitions / atoms per block
BLOCKS = 2                   # atom blocks per core (256 atoms/core)
F = 768                      # time-tile width; sawtooth periods divide F
N_TILES = T // F + (1 if T % F else 0)      # 63 (62 full + 384 remainder)
REM = T - (N_TILES - 1) * F                 # 384
CHUNK = 128                  # reduce chunk (output column) width
N_COLS = T // CHUNK          # 375 output columns
KSAW = 75                    # sawtooth one-hot rows per block (padded)
K = KSAW + 5                 # + C1, C2 (ones rows) and r1*j1, r1*j2, r2*j1
DMA_GRP = 8                  # stat DMA split: tiles per chunk

f32 = mybir.dt.float32
f16 = mybir.dt.float16
bf16 = mybir.dt.bfloat16
bft = ml_dtypes.bfloat16
TWO_PI = 2.0 * np.pi

_cache = {}


def _build_program():
    nc = bacc.Bacc("TRN2", target_bir_lowering=False, debug=False)

    d_saw = [nc.dram_tensor(f"saw{b}", [K, F], bf16, kind="ExternalInput").ap()
             for b in range(BLOCKS)]
    # stationary, k-major: [K, tile, block, atom]
    d_stat = nc.dram_tensor("stat", [K, N_TILES, BLOCKS, P], bf16,
                            kind="ExternalInput").ap()
    # envelope knots per block: value/slope per 128-sample chunk
    d_e0 = [nc.dram_tensor(f"e0_{b}", [P, N_COLS], bf16,
                           kind="ExternalInput").ap() for b in range(BLOCKS)]
    d_de = [nc.dram_tensor(f"de_{b}", [P, N_COLS], bf16,
                           kind="ExternalInput").ap() for b in range(BLOCKS)]
    d_prow = nc.dram_tensor("prow", [P, 1], f32, kind="ExternalInput").ap()
    d_out = nc.dram_tensor("wave", [P, N_COLS], f32, kind="ExternalOutput").ap()

    with tile.TileContext(nc) as tc, ExitStack() as ctx:
        consts = ctx.enter_context(tc.tile_pool(name="consts", bufs=1))
        phpool = ctx.enter_context(tc.tile_pool(name="ph", bufs=8))
        cospool = ctx.enter_context(tc.tile_pool(name="cos", bufs=14))
        yfpool = ctx.enter_context(tc.tile_pool(name="yf", bufs=2))
        kpool = ctx.enter_context(tc.tile_pool(name="k2pi", bufs=2))
        opool = ctx.enter_context(tc.tile_pool(name="ocopy", bufs=1))
        ypool = ctx.enter_context(tc.tile_pool(name="yp", bufs=2, space="PSUM"))
        outpool = ctx.enter_context(tc.tile_pool(name="outp", bufs=1,
                                                 space="PSUM"))

        # ---- resident constants (queues spread so fixed DMA overheads
        # overlap during the pipeline ramp) ----
        t_saw = []
        for b in range(BLOCKS):
            t = consts.tile([K, F], bf16, tag=f"saw{b}")
            nc.sync.dma_start(t[:], d_saw[b][:])
            t_saw.append(t)
        t_e0, t_de = [], []
        for b in range(BLOCKS):
            te = consts.tile([P, N_COLS], bf16, tag=f"e0_{b}")
            nc.gpsimd.dma_start(te[:], d_e0[b][:])
            t_e0.append(te)
            td = consts.tile([P, N_COLS], bf16, tag=f"de_{b}")
            nc.gpsimd.dma_start(td[:], d_de[b][:])
            t_de.append(td)
        t_prow = consts.tile([P, 1], f32, tag="prow")
        nc.gpsimd.dma_start(t_prow[:], d_prow[:])

        # all per-tile stationaries, resident. The first chunk covers only
        # the first tiles processed (fast pipeline start), the rest follow
        # in DMA_GRP-tile chunks.
        t_stat = consts.tile([K, N_TILES * BLOCKS * P], bf16, tag="stat")

        def stat_dma(lo, hi):
            nc.sync.dma_start(
                t_stat[:, lo * BLOCKS * P: hi * BLOCKS * P],
                d_stat[:, lo:hi])

        stat_dma(N_TILES - 1, N_TILES)       # REM tile, processed first
        stat_dma(0, 2)
        i = 2
        while i < N_TILES - 1:
            hi = min(i + DMA_GRP, N_TILES - 1)
            stat_dma(i, hi)
            i = hi

        p_out = outpool.tile([P, 512], f32, tag="po")
        q_out = outpool.tile([P, 512], f32, tag="qo")

        def tf(i):
            return REM if i == N_TILES - 1 else F

        def spans_of(w):
            return ([slice(0, BLOCKS * F)] if w == F else
                    [slice(b * F, b * F + w) for b in range(BLOCKS)])

        # two-stage software pipeline: stage_a(i) produces the cosine tile;
        # the weighted reduce runs one tile behind so PE queue heads always
        # have ready deps. Quarter-angle tiles skip the DVE wrap entirely:
        # Sin(0.25*y) is always in range (|y|/4 <= ~2 < pi), and
        # cos(y) = 2*(1 - 2*sin^2(y/4))^2 - 1 reconstructs via two ACT
        # Squares (same table set as Sin) staged over later iterations plus
        # two cheap 4x DVE tensor_scalars.
        stash = {}
        qstate = {}
        QUARTER_TILES = frozenset()

        def stage_a(i):
            w = tf(i)
            p_y = ypool.tile([P, BLOCKS * F], f32, tag="y")
            for b in range(BLOCKS):
                st = t_stat[:, (i * BLOCKS + b) * P: (i * BLOCKS + b + 1) * P]
                o = 0
                while o < w:
                    col = b * F + o
                    n = min(w - o, 512 - (col % 512))
                    nc.tensor.matmul(p_y[:, col: col + n],
                                     st, t_saw[b][:, o:o + n],
                                     start=True, stop=True)
                    o += n
            t_cos = cospool.tile([P, BLOCKS * F], bf16, tag="cos")
            if i in QUARTER_TILES:
                t_s = yfpool.tile([P, BLOCKS * F], f16, tag="qs")
                for sl in spans_of(w):
                    nc.scalar.activation(t_s[:, sl], p_y[:, sl],
                                         mybir.ActivationFunctionType.Sin,
                                         scale=0.25)
                qstate[i] = (t_s, t_cos, w)
            else:
                # phase wrap into [-pi, pi], PSUM -> SBUF f16 so the PSUM
                # tile frees at the wrap itself and the pipeline runs at
                # wrap rate (one period is enough: host centering keeps
                # |y| <= ~7.9 < 3*pi)
                t_ph = phpool.tile([P, BLOCKS * F], f16, tag="ph")
                for sl in spans_of(w):
                    nc.vector.add_range_wrap(t_ph[:, sl], p_y[:, sl],
                                             0.0, np.pi, TWO_PI)
                    nc.scalar.activation(t_cos[:, sl], t_ph[:, sl],
                                         mybir.ActivationFunctionType.Sin)
            stash[i] = t_cos

        def stage_q1(i):
            t_s, t_cos, w = qstate[i]
            t_g = kpool.tile([P, BLOCKS * F], f16, tag="qg")
            for sl in spans_of(w):
                nc.scalar.activation(t_s[:, sl], t_s[:, sl],
                                     mybir.ActivationFunctionType.Square)
                nc.vector.tensor_scalar(t_g[:, sl], t_s[:, sl], -2.0, 1.0,
                                        mybir.AluOpType.mult,
                                        mybir.AluOpType.add)
            qstate[i] = (t_g, t_cos, w)

        def stage_q2(i):
            t_g, t_cos, w = qstate.pop(i)
            for sl in spans_of(w):
                nc.scalar.activation(t_g[:, sl], t_g[:, sl],
                                     mybir.ActivationFunctionType.Square)
                nc.vector.tensor_scalar(t_cos[:, sl], t_g[:, sl], 2.0, -1.0,
                                        mybir.AluOpType.mult,
                                        mybir.AluOpType.add)

        def stage_reduce(i):
            w = tf(i)
            t_cos = stash.pop(i)
            for j in range(w // CHUNK):
                c = (i * F) // CHUNK + j
                for dst, mov in ((p_out, t_e0), (q_out, t_de)):
                    for b in range(BLOCKS):
                        nc.tensor.matmul(
                            dst[:, c:c + 1],
                            t_cos[:, b * F + j * CHUNK: b * F + (j + 1) * CHUNK],
                            mov[b][:, c:c + 1],
                            start=(b == 0), stop=(b == BLOCKS - 1))

        # process the small REM tile first: faster pipeline fill and the
        # run ends on a predictable full tile. Quarter tiles are swapped
        # ahead of their predecessor so their PSUM-reading Sin heads ACT's
        # queue and frees the PSUM slot without waiting a tile of Sins.
        order = [N_TILES - 1] + list(range(N_TILES - 1))
        for q in QUARTER_TILES:
            k = order.index(q)
            order[k - 1], order[k] = order[k], order[k - 1]
        t_t = opool.tile([P, N_COLS], f32, tag="tmp")
        t_w = opool.tile([P, N_COLS], f32, tag="w")

        def combine(c_lo, c_hi):
            # wave = p_out + prow*q_out (env = e0 + p*de within each chunk)
            cs = slice(c_lo, c_hi)
            nc.vector.tensor_scalar(t_t[:, cs], q_out[:, cs], t_prow[:],
                                    None, mybir.AluOpType.mult)
            nc.vector.tensor_tensor(t_w[:, cs], t_t[:, cs], p_out[:, cs],
                                    mybir.AluOpType.add)
            nc.sync.dma_start(d_out[:, cs], t_w[:, cs])

        for k, i in enumerate(order):
            stage_a(i)
            if k >= 2 and order[k - 2] in QUARTER_TILES:
                stage_q1(order[k - 2])
            if k >= 4 and order[k - 4] in QUARTER_TILES:
                stage_q2(order[k - 4])
            if k >= 3 and order[k - 3] not in QUARTER_TILES:
                stage_reduce(order[k - 3])
            if k >= 6 and order[k - 6] in QUARTER_TILES:
                stage_reduce(order[k - 6])
            if k == 40:
                # tiles 0..29 (cols 0..179) are reduced by now: overlap the
                # first chunk of the final combine + output DMA
                combine(0, 180)
        for k in range(len(order) - 3, len(order)):
            if order[k] in stash:
                stage_reduce(order[k])
        assert not stash and not qstate
        combine(180, N_COLS)

    nc.compile()
    return nc


def _cfrac(x):
    return x - np.round(x)


def _prepare_inputs(amplitude_logit, tau, omega_logit, sigma_logit,
                    phi_vector, gamma):
    """fp64 host prep -> per-core input maps."""
    al = amplitude_logit.astype(np.float64)
    tau = tau.astype(np.float64)
    ol = omega_logit.astype(np.float64)
    sl = sigma_logit.astype(np.float64)
    pv = phi_vector.astype(np.float64)
    gamma = gamma.astype(np.float64)

    amp = np.where(al > 30, al, np.log1p(np.exp(al)))
    omega = (1.0 / (1.0 + np.exp(-ol))) * 0.99 * NYQUIST
    sigma = np.where(sl > 30, sl, np.log1p(np.exp(sl))) + SIGMA_OFFSET
    phi = np.arctan2(pv[:, 1], pv[:, 0])

    # sort atoms by center phase rate (cycles/sample); deal runs of 256/core
    beta_mid = (omega + gamma * (1.0 - tau) / np.pi) / FS
    order = np.argsort(beta_mid)
    amp, tau_s, omega_s = amp[order], tau[order], omega[order]
    sigma_s, phi_s, gamma_s = sigma[order], phi[order], gamma[order]
    m_all = np.round(F * beta_mid[order]).astype(np.int64)

    jl = np.arange(F, dtype=np.float64)
    t0s = np.arange(N_TILES, dtype=np.float64) * F / FS        # [I]
    kn = np.arange(N_COLS + 1, dtype=np.float64) * CHUNK / FS  # chunk knots
    prow = np.arange(P, dtype=np.float32).reshape(P, 1)

    in_maps = []
    for c in range(N_CORES):
        saws, e0s, des = [], [], []
        stat = np.zeros((K, N_TILES, BLOCKS, P), dtype=bft)
        for b in range(BLOCKS):
            sel = slice(c * BLOCKS * P + b * P, c * BLOCKS * P + (b + 1) * P)
            am, ta, om = amp[sel], tau_s[sel], omega_s[sel]
            sg, ph, ga = sigma_s[sel], phi_s[sel], gamma_s[sel]
            m = m_all[sel]

            ms = np.unique(m)
            ms = ms[ms != 0]
            assert len(ms) <= KSAW, f"block saw rows {len(ms)} > {KSAW}"
            saw = np.zeros((K, F), dtype=bft)
            saw[:len(ms)] = (TWO_PI * _cfrac(ms[:, None] * jl[None, :] / F)
                             ).astype(bft)
            # residual-rate moving rows: ones, ones, j1, j2, j1
            j1 = jl.astype(bft)
            j2 = (jl - j1.astype(np.float64)).astype(bft)
            saw[KSAW + 0] = bft(1.0)
            saw[KSAW + 1] = bft(1.0)
            saw[KSAW + 2] = j1
            saw[KSAW + 3] = j2
            saw[KSAW + 4] = j1
            saws.append(np.ascontiguousarray(saw))

            row_of = {mm: r for r, mm in enumerate(ms)}
            rows = np.array([row_of.get(mm, -1) for mm in m])  # [P]

            # per (tile, atom): phase at tile start, rate, residual
            D = t0s[:, None] - ta[None, :]                      # [I, P]
            Y0 = (om[None, :] * D + ga[None, :] * D * D / TWO_PI
                  + ph[None, :] / TWO_PI + 0.25)               # cycles
            beta_t = (om[None, :] + ga[None, :] * D / np.pi) / FS
            r = beta_t - m[None, :] / F                        # [I, P]
            assert np.abs(r).max() < 1.0 / 1536 + 1e-5

            C_raw = _cfrac(Y0)
            mid = C_raw + r * (F - 1) / 2.0
            Cc = C_raw - np.round(mid)                         # center |y|
            C_rad = TWO_PI * Cc
            C1 = C_rad.astype(bft)
            C2 = (C_rad - C1.astype(np.float64)).astype(bft)
            r_rad = TWO_PI * r
            r1 = r_rad.astype(bft)
            r2 = (r_rad - r1.astype(np.float64)).astype(bft)

            onehot = np.zeros((KSAW, P), dtype=bft)
            pidx = np.nonzero(rows >= 0)[0]
            onehot[rows[pidx], pidx] = bft(1.0)
            stat[:KSAW, :, b, :] = onehot[:, None, :]
            stat[KSAW + 0, :, b, :] = C1
            stat[KSAW + 1, :, b, :] = C2
            stat[KSAW + 2, :, b, :] = r1
            stat[KSAW + 3, :, b, :] = r1
            stat[KSAW + 4, :, b, :] = r2

            # envelope knots at chunk boundaries -> e0, de per chunk
            dk = kn[:, None] - ta[None, :]                      # [C+1, P]
            ev = am[None, :] * np.exp(-0.5 * (dk / sg[None, :]) ** 2)
            e0 = ev[:-1]                                        # [C, P]
            de = (ev[1:] - ev[:-1]) / CHUNK
            e0s.append(np.ascontiguousarray(e0.T.astype(bft)))
            des.append(np.ascontiguousarray(de.T.astype(bft)))

        im = {"stat": np.ascontiguousarray(stat),
              "prow": prow}
        for b in range(BLOCKS):
            im[f"saw{b}"] = saws[b]
            im[f"e0_{b}"] = e0s[b]
            im[f"de_{b}"] = des[b]
        in_maps.append(im)
    return in_maps


def kernel(amplitude_logit, tau, omega_logit, sigma_logit, phi_vector, gamma, t):
    if "nc" not in _cache:
        _cache["nc"] = _build_program()
    nc = _cache["nc"]
    in_maps = _prepare_inputs(amplitude_logit, tau, omega_logit, sigma_logit,
                              phi_vector, gamma)
    res = run_bass_kernel_spmd(nc, in_maps, list(range(N_CORES)))
    total = np.zeros(T, dtype=np.float64)
    for r in res.results:
        wv = r["wave"].astype(np.float64)          # [P, N_COLS]
        total += wv.T.ravel()                      # sample s = c*128 + p
    return total.astype(np.float32)
